# revision 25
# baseline (speedup 1.0000x reference)
"""Trainium2 Bass kernel for nn_AttentionCrossLayer.

Math: in the reference, softmax over a length-1 axis is exactly 1.0, so
attn == v and q/k/wq/wk are dead code. With x0 the (never-mutated) input,
each layer's gate xw_i = out_i @ cw_i is a fixed linear function of x0:
    xw_i = x0 @ u_i + c_i,   u_i = Wv_i @ (Wo_i @ cw_i),
                             c_i = (bv_i @ Wo_i + bo_i) @ cw_i
and the layer recurrence x += x0 * xw_i + cb_i telescopes to
    out[b, d] = x0[b, d] * (x0[b, :] @ usum + cprime) + cbsum[d]
with usum = sum_i u_i  [D], cprime = 1 + sum_i c_i, cbsum = sum_i cb_i [D].

The tiny weight contractions happen host-side in float64. The device
kernel is one pass over x per core, 32 row-tiles of [128, 1024]:
  pass 1 (Vector): fused multiply + row-reduce (scalar_tensor_tensor
    with accum_out) -> per-row gate t. cprime rides in a constant
    column appended to x/u so the reduce emits the finished gate.
  pass 2: in-place x <- x * t + cbsum. When cbsum == 0 (the spec fills
    cb with zeros) this is a per-row scale on the Scalar engine
    (activation per-partition scale AP); a Vector path handles
    cbsum != 0.

Schedule (evidence from per-queue trace A/B over five variants):
  - Two HWDGE load rings (sync+scalar) together sustain the ~420 GB/s
    HBM rate from the first microseconds; SWDGE loads concurrent with
    any other ring degrade the pool to ~300-345 GB/s, so gpsimd never
    loads. SWDGE stores sustain 420 GB/s alone or mixed.
  - An HWDGE ring BLOCKS its issuing engine once the ring backs up, so
    scalar interleaves its load issues with the pass-2 muls (a blocked
    issue loop would starve cm2 and with it the store stream).
  - u arrives host-replicated to [128, D] and streams in as sync's
    first (contiguous) load — no HBM-latency-bound replication
    descriptors polluting the load stream (that cost the old version
    ~15us of ramp), no slow fp32 PE path.
  - One semaphore per store: the DGE serializes DMAs that share a
    semaphore (~2.5us/DMA), which caps a shared-sem store stream at
    ~200 GB/s. Final wait is a chain of per-store waits.
  - Block(no_gpsimd_drain=True): skips a ~3.6us GpSimd dge_drain in
    the epilogue; the store-sem waits already guarantee completion.

Sharding: data-parallel over batch across 8 cores, weights replicated,
no cross-device comms.
"""

import numpy as np

L, B, D, H, K = 3, 32768, 1024, 8, 64
N_CORES = 8
B_LOC = B // N_CORES  # 4096 rows per core
P = 128
N_TILES = B_LOC // P  # 32
DP = D + 32  # slot stride 4224B = 128B aligned; col D holds the 1.0 constant

_cache = {}


def _build_program(cprime: float, zero_cb: bool):
    import concourse.bass as bass
    from concourse import mybir

    F32 = mybir.dt.float32
    BF16 = mybir.dt.bfloat16
    MUL = mybir.AluOpType.mult
    ADD = mybir.AluOpType.add

    nc = bass.Bass()
    x = nc.declare_dram_parameter("x", [B_LOC, D], F32, isOutput=False)
    # u arrives host-replicated to [P, D+1] bf16 (262KB) with the
    # cprime column baked in: one plain contiguous streaming load —
    # no HBM-latency-bound replication descriptors (those poisoned the
    # whole DMA pool for ~15us) and no slow fp32 PE broadcast. bf16 is
    # fine: |u| errors ~0.4% against a 2e-2 relative budget, and the
    # DVE converts mixed-dtype STT operands on read.
    u = nc.declare_dram_parameter("u", [P, D + 1], BF16, isOutput=False)
    cb = nc.declare_dram_parameter("cb", [1, D], F32, isOutput=False)
    out = nc.declare_dram_parameter("out", [B_LOC, D], F32, isOutput=True)

    cb_bcast = bass.AP(tensor=cb.ap().tensor, offset=0, ap=[[0, P], [1, D]])

    # sync: evens plus 31; scalar: odds 1..29. Balanced so both load
    # rings drain their queues at ~the same time (scalar's starts a
    # beat later and pays a small solo-rate tail), since the
    # last-arriving tile gates the pass1->pass2->store chain tail.
    sy_tiles = [i for i in range(N_TILES) if i % 2 == 0] + [31]
    sy_tiles.sort()
    sc_tiles = [i for i in range(1, 30) if i % 2 == 1]  # scalar: odds 1..29

    with (
        nc.sbuf_tensor([P, D + 1], BF16) as ub,  # [:, :D]=usum, [:, D]=cprime
        nc.sbuf_tensor([P, D], F32) as cbb,
        nc.sbuf_tensor([P, N_TILES, DP], F32) as xt,  # [:, i, D] = 1.0
        nc.sbuf_tensor([P, 2, D + 1], F32) as oscr,  # throwaway STT main out
        nc.sbuf_tensor([P, N_TILES, 1], F32) as tsc,
        nc.semaphore("ubb") as ubb,  # u block landed (sync ring)
        nc.semaphore("cbs") as cbs,  # cb broadcast landed (general path)
        nc.semaphore("cm") as cm,  # pass-1 reduces retired (Vector)
        nc.semaphore("cm2") as cm2,  # pass-2 writes retired
        nc.Block(no_gpsimd_drain=True) as block,
    ):
        lds = [nc.alloc_semaphore(f"ld{i}") for i in range(N_TILES)]
        sts = [nc.alloc_semaphore(f"st{i}") for i in range(N_TILES)]

        @block.sync
        def _(sync):
            # u block first: lands by ~9.5us, gating only pass 1.
            sync.dma_start(out=ub[:, :], in_=u.ap()).then_inc(ubb, 16)
            for i in sy_tiles:
                sync.dma_start(
                    out=xt[:, i, 0:D], in_=x[i * P : (i + 1) * P, :]
                ).then_inc(lds[i], 16)

        @block.scalar
        def _(scalar):
            if not zero_cb:
                scalar.dma_start(out=cbb[:, :], in_=cb_bcast).then_inc(cbs, 16)
            # tiles 1,3 up front (ring is empty, issues don't block);
            # the rest interleave with the pass-2 muls below so a
            # backed-up ring can never starve cm2.
            head, rest = sc_tiles[:2], sc_tiles[2:]
            for i in head:
                scalar.dma_start(
                    out=xt[:, i, 0:D], in_=x[i * P : (i + 1) * P, :]
                ).then_inc(lds[i], 16)
            if zero_cb:
                for i in range(N_TILES):
                    if i < len(rest):
                        j = rest[i]
                        scalar.dma_start(
                            out=xt[:, j, 0:D], in_=x[j * P : (j + 1) * P, :]
                        ).then_inc(lds[j], 16)
                    scalar.wait_ge(cm, i + 1)
                    # pass 2: x <- x * t (cbsum == 0)
                    nc.scalar.mul(
                        out=xt[:, i, 0:D],
                        in_=xt[:, i, 0:D],
                        mul=tsc[:, i, :],
                    ).then_inc(cm2, 1)
            else:
                for j in rest:
                    scalar.dma_start(
                        out=xt[:, j, 0:D], in_=x[j * P : (j + 1) * P, :]
                    ).then_inc(lds[j], 16)

        @block.vector
        def _(vector):
            nc.vector.memset(xt[:, :, D : D + 1], 1.0)
            vector.wait_ge(ubb, 16)
            if not zero_cb:
                vector.wait_ge(cbs, 16)
            for i in range(N_TILES):
                vector.wait_ge(lds[i], 16)
                # oscr = x' * u' ; t_i = sum_free = x.usum + cprime
                nc.vector.scalar_tensor_tensor(
                    out=oscr[:, i % 2, :],
                    in0=xt[:, i, 0 : D + 1],
                    scalar=1.0,
                    in1=ub[:, :],
                    op0=MUL,
                    op1=MUL,
                    accum_out=tsc[:, i, :],
                ).then_inc(cm, 1)
                if not zero_cb:
                    # accumulator writeback must retire before t is read
                    vector.wait_ge(cm, i + 1)
                    # in place: x <- x * t + cbsum
                    nc.vector.scalar_tensor_tensor(
                        out=xt[:, i, 0:D],
                        in0=xt[:, i, 0:D],
                        scalar=tsc[:, i, :],
                        in1=cbb[:, :],
                        op0=MUL,
                        op1=ADD,
                    ).then_inc(cm2, 1)

        @block.gpsimd
        def _(gpsimd):
            for i in range(N_TILES):
                gpsimd.wait_ge(cm2, i + 1)
                gpsimd.dma_start(
                    out=out[i * P : (i + 1) * P, :], in_=xt[:, i, 0:D]
                ).then_inc(sts[i], 16)
            # No final store-completion waits: the program epilogue (NRT
            # postamble) runs concurrently with the trailing stores, and
            # the runtime quiesces DMA before execution completes, so
            # the output is fully in HBM before the host can read it.

    return nc


def _precompute(wv, bv, wo, bo, cw, cb):
    """Host-side f64 contraction of the small per-layer weights."""
    usum = np.zeros(D, np.float64)
    cprime = 1.0
    for i in range(L):
        Wv = wv[i].reshape(D, H * K).astype(np.float64)
        Wo = wo[i].reshape(H * K, D).astype(np.float64)
        cwi = cw[i].reshape(D).astype(np.float64)
        wocw = Wo @ cwi
        usum += Wv @ wocw
        cprime += float(bv[i].reshape(H * K).astype(np.float64) @ wocw)
        cprime += float(bo[i].astype(np.float64) @ cwi)
    cbsum = cb.astype(np.float64).sum(axis=0)
    return usum.astype(np.float32), float(np.float32(cprime)), cbsum.astype(np.float32)


def _ensure_trace_hook_importable():
    # bass_utils unconditionally imports antenv.axon_hooks when the
    # BASS_TRACE env var is set; some images lack that module. A None
    # hook makes bass_utils skip tracing gracefully.
    try:
        import antenv.axon_hooks  # noqa: F401
    except ImportError:
        import sys
        import types

        mod = types.ModuleType("antenv.axon_hooks")
        mod.get_axon_ntff_profile_hook = lambda: None
        mod.set_axon_ntff_profile_hook = lambda hook: None
        sys.modules["antenv.axon_hooks"] = mod


def kernel(x, wq, bq, wk, bk, wv, bv, wo, bo, cw, cb):
    from concourse.bass_utils import run_bass_kernel_spmd

    _ensure_trace_hook_importable()

    x = np.ascontiguousarray(np.asarray(x, dtype=np.float32))
    usum, cprime, cbsum = _precompute(
        np.asarray(wv), np.asarray(bv), np.asarray(wo), np.asarray(bo),
        np.asarray(cw), np.asarray(cb),
    )
    zero_cb = not np.any(cbsum)

    key = (cprime, zero_cb)
    if key not in _cache:
        _cache[key] = _build_program(cprime, zero_cb)
    nc = _cache[key]

    import ml_dtypes

    urow = np.concatenate([usum, np.float32(cprime).reshape(1)]).astype(
        ml_dtypes.bfloat16
    )
    u2 = np.ascontiguousarray(np.broadcast_to(urow.reshape(1, D + 1), (P, D + 1)))
    cb2 = cbsum.reshape(1, D)
    in_maps = [
        {"x": x[c * B_LOC : (c + 1) * B_LOC], "u": u2, "cb": cb2}
        for c in range(N_CORES)
    ]
    res = run_bass_kernel_spmd(nc, in_maps, list(range(N_CORES)))
    return np.concatenate([res.results[c]["out"] for c in range(N_CORES)], axis=0)
